# revision 37
# baseline (speedup 1.0000x reference)
"""Causal self-attention (B=2, T=2048, C=1024, H=16) on 8 TRN2 NeuronCores.

Sharding: data-parallel over batch x tensor-parallel over heads.
Core c handles batch c//4 and the 4 heads (c%4)*4 .. (c%4)*4+3.

v2 design (vs v1 baseline):
  - x is transposed on the host; the kernel receives x^T [C, T] directly,
    eliminating all PE transposes and their PSUM->SBUF evacuations.
  - The per-rep schedule is a software pipeline over 512-row chunks ci:
    QKV(ci+1) and PROJ(ci-1) matmul groups are interleaved into the
    ATTN(ci) instruction stream so the PE never drains while the scalar
    engine works through the exp()s (attention is ACT-bound per block).
  - ST matmuls for the two heads of a pair sit in disjoint 64-row groups
    of the PE array (tile_position auto-derived from base_partition) and
    are issued back-to-back, so they execute concurrently on hardware.
  - exp() runs once per (pair, j-block) on a [128, 2x512] PSUM region via
    a 3D access pattern, halving activation-instruction overhead.
  - The causal mask multiplies only the 128-col diagonal triangle block,
    in place, with a host-precomputed doubled [tri|tri] tile.
  - The softmax-denominator ones-column of V_aug is memset once at setup;
    the attn bias (zero in this problem) folds into an extra contraction
    block only when nonzero.
  - Explicit Ldweights instructions are folded back into self-loading
    matmuls (fuse_ldweights) and walrus ldw-opt is re-enabled via a
    driver shim, shrinking the PE instruction stream.

Matmuls run in bf16 / float32r (~1.5e-4 rel err, 4x the fp32 rate).
"""
import sys
sys.path.insert(0, '/opt/trn_rl_repo')

from contextlib import ExitStack

import numpy as np

import concourse.bass as bass
import concourse.tile as tile
from concourse import mybir

B, T, C, H, HD = 2, 2048, 1024, 16, 64
N_CORES = 8
HPC = H // (N_CORES // B)     # heads per core = 4
CPH = HPC * HD                # channel slice per core = 256

f32 = mybir.dt.float32
f32r = mybir.dt.float32r
bf16 = mybir.dt.bfloat16
AF = mybir.ActivationFunctionType

NI = T // 512   # 4 chunks of 512 rows
NCB = C // 128  # 8 contraction blocks

# ---------------------------------------------------------------------------
# Workaround for this container's walrus codegen, which rejects instructions
# carrying more than one sync-wait command ("Too many sync wait commands").
# After Tile scheduling, hoist excess waits onto same-engine NoOps inserted
# immediately before the owning instruction (engine streams are sequential,
# so this preserves semantics exactly).
# ---------------------------------------------------------------------------
import concourse.tile as tile_mod
from bass_rust import ScopedClock, SyncInfo

MAX_WAITS = 1


def _drain_and_barrier(self, tick_clock, wait_clock):
    nc = self.nc
    drain_inst = nc.sync.drain()
    wait_clock.add_sem_waits(
        drain_inst.ins, ScopedClock({None: tick_clock.global_clock})
    )
    si = drain_inst.ins.sync_info
    if si is not None and len(si.on_wait) > MAX_WAITS:
        waits = list(si.on_wait)
        drain_inst.ins.sync_info = SyncInfo(
            on_wait=waits[:MAX_WAITS], on_update=list(si.on_update)
        )
        for k in range(MAX_WAITS, len(waits), MAX_WAITS):
            nop = nc.sync.nop(nofuse=True)
            nop.ins.sync_info = SyncInfo(on_wait=waits[k:k + MAX_WAITS], on_update=[])
    nc.all_engine_barrier()
    assert self.sems is not None
    popped = nc._tile_sem_poison_stack.pop()
    assert popped is self._sem_poison
    nc.clear_and_free_semaphores(list(self.sems.allocated().values()))
    nc.all_engine_barrier()


tile_mod.TileContext._drain_and_barrier = _drain_and_barrier

# ---------------------------------------------------------------------------
# The stock bass compile path hardcodes --enable-ldw-opt=false.  With the
# explicit Ldweights folded into self-loading matmuls (fuse_ldweights
# below), re-enabling ldw-opt lets walrus elide repeated identical weight
# loads (verified by NEFF disasm) and halves the PE instruction count.
# Controlled by KERNEL_LDW_OPT (default on) for A/B measurement.
# ---------------------------------------------------------------------------
import os as _os
import stat as _stat

_LDWOPT_SENTINEL = "/tmp/ldwopt_on"

if _os.environ.get("KERNEL_LDW_OPT", "1") == "1":
    import concourse.bass_utils as _bass_utils_mod

    if getattr(_bass_utils_mod.get_walrus_driver, "_is_ldw_shim", False):
        _orig_get_walrus_driver = _bass_utils_mod.get_walrus_driver._orig
    else:
        _orig_get_walrus_driver = _bass_utils_mod.get_walrus_driver

    def _shimmed_walrus_driver():
        real = _orig_get_walrus_driver()
        shim = "/tmp/walrus_ldwopt_shim.sh"
        with open(shim, "w") as f:
            f.write(
                "#!/bin/sh\n"
                f'if [ -f {_LDWOPT_SENTINEL} ]; then\n'
                'args=""\n'
                'for a in "$@"; do\n'
                '  case "$a" in\n'
                "    --enable-ldw-opt=false) a=--enable-ldw-opt=true;;\n"
                "  esac\n"
                '  args="$args \\"$a\\""\n'
                "done\n"
                f'eval exec "{real}" $args\n'
                "else\n"
                f'exec "{real}" "$@"\n'
                "fi\n"
            )
        _os.chmod(shim, _os.stat(shim).st_mode | _stat.S_IEXEC)
        return shim

    _shimmed_walrus_driver._is_ldw_shim = True
    _shimmed_walrus_driver._orig = _orig_get_walrus_driver
    _bass_utils_mod.get_walrus_driver = _shimmed_walrus_driver


def ldwopt_on():
    with open(_LDWOPT_SENTINEL, "w") as f:
        f.write("1")


def ldwopt_off():
    if _os.path.exists(_LDWOPT_SENTINEL):
        _os.remove(_LDWOPT_SENTINEL)


if _os.environ.get("KERNEL_LDW_OPT", "1") == "1":
    ldwopt_on()

_split_counter = [0]


def fuse_ldweights(nc):
    """Fold explicit InstLdweights into their Matmult (self-loading form).

    The tile scheduler splits each matmul into Ldweights+Matmult, but this
    container's walrus runs with LDW optimization rejected for explicit
    Ldweights, so every weight load serializes with the matmul stream
    (~100ns each).  Deleting the Ldweights and setting ldweights=True on the
    Matmult hands weight-load scheduling back to walrus codegen, whose
    ldw-opt (re-enabled via the driver shim) pipelines loads into the
    background weight buffer.  The Ldweights' sem waits/updates move onto
    the Matmult: waits still gate the weight read; updates fire at matmul
    completion, which only lengthens the protected window (safe).
    """
    n = 0
    for f in nc.m.functions:
        for bb in f.blocks:
            il = bb.instructions
            out = []
            pending = None
            for ins in il:
                if ins.opcode == "Ldweights":
                    assert pending is None, "two Ldweights without a Matmult"
                    pending = ins
                    continue
                if (pending is not None
                        and ins.engine == mybir.EngineType.PE
                        and ins.opcode != "Matmult"):
                    # unexpected PE instruction between the pair: keep the
                    # explicit Ldweights rather than fusing across it
                    out.append(pending)
                    pending = None
                if (pending is not None
                        and ins.engine == mybir.EngineType.PE):
                    lsi, msi = pending.sync_info, ins.sync_info
                    waits = list(lsi.on_wait) if lsi else []
                    upds = list(lsi.on_update) if lsi else []
                    if msi is not None:
                        waits += list(msi.on_wait)
                        upds += list(msi.on_update)
                    ins.sync_info = SyncInfo(on_wait=waits, on_update=upds)
                    ins.ldweights = True
                    pending = None
                    n += 1
                out.append(ins)
            if pending is not None:
                out.append(pending)
            if len(out) != len(il):
                il[:] = out
    return n


def split_excess_waits(nc, max_waits=MAX_WAITS):
    n_split = 0
    for f in nc.m.functions:
        for bb in f.blocks:
            il = bb.instructions
            out = []
            for ins in il:
                si = ins.sync_info
                if si is not None and len(si.on_wait) > max_waits:
                    waits = list(si.on_wait)
                    extra = waits[:-max_waits]
                    for k in range(0, len(extra), max_waits):
                        _split_counter[0] += 1
                        nop = mybir.InstNoOp(
                            name=f"wsplit-{_split_counter[0]}", ins=[], outs=[]
                        )
                        nop.engine = ins.engine
                        nop.sync_info = SyncInfo(
                            on_wait=extra[k:k + max_waits], on_update=[]
                        )
                        out.append(nop)
                    ins.sync_info = SyncInfo(
                        on_wait=waits[-max_waits:], on_update=list(si.on_update)
                    )
                    n_split += 1
                out.append(ins)
            if len(out) != len(il):
                il[:] = out
    return n_split


# ---------------------------------------------------------------------------
# Program builder
# ---------------------------------------------------------------------------
def build_program(reps=1, split_waits=True, use_bias=False, only=None,
                  fuse_ldw=None):
    nc = bass.Bass("TRN2", target_bir_lowering=False, debug=False)

    xt_d = nc.dram_tensor("xt", [C, T], bf16, kind="ExternalInput")
    wqk_d = nc.dram_tensor("wqk", [C, 512], bf16, kind="ExternalInput")
    wv_d = nc.dram_tensor("wv", [C, CPH], bf16, kind="ExternalInput")
    wp_d = nc.dram_tensor("wp", [CPH, C], bf16, kind="ExternalInput")
    masks_d = nc.dram_tensor("masks", [128, 256], bf16, kind="ExternalInput")
    if use_bias:
        w9_d = nc.dram_tensor("w9", [128, 768], bf16, kind="ExternalInput")
    y_d = nc.dram_tensor("y", [T, C], bf16, kind="ExternalOutput")

    with tile.TileContext(nc) as tc:
        with ExitStack() as ctx:
            const = ctx.enter_context(tc.tile_pool(name="const", bufs=1))
            tri2 = const.tile([128, 256], bf16, tag="tri2")
            nc.sync.dma_start(tri2[:], masks_d.ap())
            # ones row for broadcasting reciprocal rows across partitions
            tones_f = const.tile([1, 64], f32, tag="tones_f")
            nc.gpsimd.memset(tones_f[:], 1.0)
            tones = const.tile([1, 64], f32r, tag="tones")
            nc.vector.tensor_copy(tones[:], tones_f[:])
            if use_bias:
                x9 = const.tile([128, 512], bf16, tag="x9")
                nc.gpsimd.memset(x9[0:1, :], 1.0)
                nc.gpsimd.memset(x9[1:128, :], 0.0)
                w9_t = const.tile([128, 768], bf16, tag="w9")
                nc.sync.dma_start(w9_t[:], w9_d.ap())

            # persistent data tiles (reused every rep)
            pers = ctx.enter_context(tc.tile_pool(name="pers", bufs=1))
            qkt = [pers.tile([128, T], bf16, tag=f"qkt{m}", name=f"qkt{m}")
                   for m in range(4)]
            vaug = [pers.tile([128, HPC * 65], bf16, tag=f"va{tb}", name=f"va{tb}")
                    for tb in range(T // 128)]
            for tb in range(T // 128):
                nc.gpsimd.memset(
                    vaug[tb][:].rearrange("p (h e) -> p h e", e=65)[:, :, 64:65],
                    1.0)
            yts = [pers.tile([128, T], bf16, tag=f"yts{k}", name=f"yts{k}")
                   for k in range(2)]
            wpt = [pers.tile([128, C], bf16, tag=f"wp{kb}", name=f"wpt{kb}")
                   for kb in range(2)]
            wqk_t = [pers.tile([128, 512], bf16, tag=f"wqk{cb}", name=f"wqk{cb}")
                     for cb in range(NCB)]
            wv_t = [pers.tile([128, CPH], bf16, tag=f"wv{cb}", name=f"wv{cb}")
                    for cb in range(NCB)]

            with ExitStack() as c2:
                    xp = c2.enter_context(tc.tile_pool(name="xp", bufs=2))
                    mmps = c2.enter_context(
                        tc.tile_pool(name="mmps", bufs=2, space="PSUM"))
                    sps = c2.enter_context(
                        tc.tile_pool(name="sps", bufs=2, space="PSUM"))
                    yps = c2.enter_context(
                        tc.tile_pool(name="yps", bufs=2, space="PSUM"))
                    ep = c2.enter_context(tc.tile_pool(name="ep", bufs=5))
                    rp = c2.enter_context(tc.tile_pool(name="rp", bufs=3))
                    op = c2.enter_context(tc.tile_pool(name="op", bufs=3))

                    # per-rep input DMAs
                    xts = {}

                    def dma_x(ci):
                        ts = []
                        for cb in range(NCB):
                            t = xp.tile([128, 512], bf16, tag=f"x{cb}", name=f"x{ci}_{cb}")
                            nc.sync.dma_start(
                                t[:],
                                xt_d.ap()[cb * 128:(cb + 1) * 128,
                                          ci * 512:(ci + 1) * 512])
                            ts.append(t)
                        xts[ci] = ts

                    # weights are constant across repeat-iterations: load
                    # them once at setup instead of every rep (removes the
                    # rep-boundary DMA serialization).
                    for cb in range(NCB):
                        nc.sync.dma_start(
                            wqk_t[cb][:],
                            wqk_d.ap()[cb * 128:(cb + 1) * 128, :])
                    for cb in range(NCB):
                        nc.sync.dma_start(
                            wv_t[cb][:],
                            wv_d.ap()[cb * 128:(cb + 1) * 128, :])
                    for kb in range(2):
                        nc.sync.dma_start(
                            wpt[kb][:],
                            wp_d.ap()[kb * 128:(kb + 1) * 128, :])

                    # ---- QKV for chunk ci: list of emit-thunks ------------
                    def qkv_thunks(ci):
                        thunks = []

                        def qk_group(m):
                            def emit():
                                ps = mmps.tile([128, 512], f32, tag="mm", name=f"qk{ci}_{m}")
                                for cb in range(NCB):
                                    nc.tensor.matmul(
                                        ps[:],
                                        lhsT=wqk_t[cb][:, m * 128:(m + 1) * 128],
                                        rhs=xts[ci][cb][:],
                                        start=(cb == 0),
                                        stop=(cb == NCB - 1 and not use_bias))
                                if use_bias:
                                    nc.tensor.matmul(
                                        ps[:],
                                        lhsT=w9_t[:, m * 128:(m + 1) * 128],
                                        rhs=x9[:],
                                        start=False, stop=True)
                                nc.vector.tensor_copy(
                                    qkt[m][:, ci * 512:(ci + 1) * 512], ps[:])
                            return emit

                        for m in range(4):
                            thunks.append(qk_group(m))

                        def v_group(tp_):
                            def emit():
                                ps = mmps.tile([128, 512], f32, tag="mm", name=f"v{ci}_{tp_}")
                                for sub in range(2):
                                    tloc = 2 * tp_ + sub
                                    for cb in range(NCB):
                                        nc.tensor.matmul(
                                            ps[:, sub * 256:(sub + 1) * 256],
                                            lhsT=xts[ci][cb][:, tloc * 128:
                                                             (tloc + 1) * 128],
                                            rhs=wv_t[cb][:],
                                            start=(cb == 0),
                                            stop=(cb == NCB - 1 and not use_bias))
                                    if use_bias:
                                        nc.tensor.matmul(
                                            ps[:, sub * 256:(sub + 1) * 256],
                                            lhsT=x9[:, 0:128],
                                            rhs=w9_t[:, 512:768],
                                            start=False, stop=True)
                                for sub in range(2):
                                    tb = 4 * ci + 2 * tp_ + sub
                                    nc.vector.tensor_copy(
                                        vaug[tb][:].rearrange(
                                            "p (h e) -> p h e", e=65)[:, :, 0:64],
                                        ps[:, sub * 256:(sub + 1) * 256].rearrange(
                                            "p (h d) -> p h d", d=64))
                            return emit

                        for tp_ in range(2):
                            thunks.append(v_group(tp_))
                        return thunks

                    # ---- PROJ for chunk ci: list of emit-thunks -----------
                    def proj_thunks(ci):
                        thunks = []

                        def pgroup(tb, nn_):
                            def emit():
                                ps = mmps.tile([128, 512], f32, tag="mm",
                                               name=f"p{ci}_{tb}_{nn_}")
                                for kb in range(2):
                                    nc.tensor.matmul(
                                        ps[:],
                                        lhsT=yts[kb][:, tb * 128:(tb + 1) * 128],
                                        rhs=wpt[kb][:, nn_ * 512:(nn_ + 1) * 512],
                                        start=(kb == 0), stop=(kb == 1))
                                ob = op.tile([128, 512], bf16, tag="ob", name=f"ob{tb}_{nn_}")
                                nc.vector.tensor_copy(ob[:], ps[:])
                                nc.sync.dma_start(
                                    y_d.ap()[tb * 128:(tb + 1) * 128,
                                             nn_ * 512:(nn_ + 1) * 512],
                                    ob[:])
                            return emit

                        for tb in range(4 * ci, 4 * ci + 4):
                            for nn_ in range(2):
                                thunks.append(pgroup(tb, nn_))
                        return thunks

                    # ---- ATTN for chunk ci: list of slot-thunks -----------
                    # The two head-pairs run as interleaved independent
                    # chains (pair0-bj, pair1-bj, pair0-bj+1, ...): doubles
                    # the dependency distance of the ST->exp->mask->PV chain
                    # so cross-engine semaphore latency is hidden, with the
                    # same PSUM budget (st 1x2banks, yt 4x1bank).
                    def attn_slots(ci):
                        isl = slice(ci * 512, ci * 512 + 512)
                        jmax = 4 * ci + 3
                        state = {0: {}, 1: {}}

                        def bj_slot(pair, bj):
                            qt = qkt[pair]
                            kt = qkt[2 + pair]
                            hA, hB = 2 * pair, 2 * pair + 1

                            def emit():
                                if bj == 0:
                                    state[pair]["ytA"] = yps.tile(
                                        [128, 512], f32, tag="yt",
                                        name=f"ytA{ci}_{pair}")
                                    state[pair]["ytB"] = yps.tile(
                                        [128, 512], f32, tag="yt",
                                        name=f"ytB{ci}_{pair}")
                                ytA = state[pair]["ytA"]
                                ytB = state[pair]["ytB"]
                                jsl = slice(bj * 128, bj * 128 + 128)
                                k = bj - 4 * ci
                                lo = max(k, 0) * 128
                                st = sps.tile([128, 1024], f32, tag="st",
                                              name=f"st{ci}_{pair}_{bj}")
                                nc.tensor.matmul(
                                    st[:, lo:512], lhsT=kt[0:64, jsl],
                                    rhs=qt[0:64, isl][:, lo:],
                                    start=True, stop=True)
                                nc.tensor.matmul(
                                    st[:, 512 + lo:1024], lhsT=kt[64:128, jsl],
                                    rhs=qt[64:128, isl][:, lo:],
                                    start=True, stop=True)
                                es = ep.tile([128, 1024], bf16, tag="es",
                                             name=f"es{ci}_{pair}_{bj}")
                                st3 = st[:].rearrange(
                                    "p (c i) -> p c i", c=2)[:, :, lo:]
                                es3 = es[:].rearrange(
                                    "p (c i) -> p c i", c=2)[:, :, lo:]
                                nc.scalar.activation(es3, st3, AF.Exp,
                                                     scale=0.125)
                                if k >= 0:
                                    es_tri = es[:].rearrange(
                                        "p (c i) -> p c i",
                                        c=2)[:, :, lo:lo + 128]
                                    nc.vector.tensor_mul(
                                        es_tri, es_tri,
                                        tri2[:].rearrange(
                                            "p (c i) -> p c i", c=2))
                                nc.tensor.matmul(
                                    ytA[0:65, lo:],
                                    lhsT=vaug[bj][:, hA * 65:(hA + 1) * 65],
                                    rhs=es[:, lo:512],
                                    start=(bj == 0), stop=(bj == jmax))
                                nc.tensor.matmul(
                                    ytB[0:65, lo:],
                                    lhsT=vaug[bj][:, hB * 65:(hB + 1) * 65],
                                    rhs=es[:, 512 + lo:1024],
                                    start=(bj == 0), stop=(bj == jmax))
                            return emit

                        def norm_slot(pair):
                            def emit():
                                ytA = state[pair]["ytA"]
                                ytB = state[pair]["ytB"]
                                for sub, yt_h in ((0, ytA), (1, ytB)):
                                    rc = rp.tile([1, 512], f32r, tag="rc",
                                                 name=f"rc{ci}_{pair}_{sub}")
                                    with nc.allow_low_precision(
                                            reason="f32r operand for bc mm"):
                                        nc.vector.reciprocal(rc[:],
                                                             yt_h[64:65, :])
                                    bc = mmps.tile([128, 512], f32, tag="mm",
                                                   name=f"bc{ci}_{pair}_{sub}")
                                    nc.tensor.matmul(bc[0:64, :],
                                                     lhsT=tones[:], rhs=rc[:],
                                                     start=True, stop=True)
                                    bs = rp.tile([64, 512], f32, tag="bs",
                                                 name=f"bs{ci}_{pair}_{sub}")
                                    nc.vector.tensor_copy(bs[:], bc[0:64, :])
                                    prow = slice(sub * 64, sub * 64 + 64)
                                    nc.vector.tensor_mul(
                                        yts[pair][prow, isl], yt_h[0:64, :],
                                        bs[:])
                            return emit

                        slots = []
                        for pair in range(2):
                            for bj in range(jmax + 1):
                                slots.append(bj_slot(pair, bj))
                            slots.append(norm_slot(pair))
                        return slots

                    # ---- emission schedules -------------------------------
                    def body_full():
                        dma_x(0)
                        for th in qkv_thunks(0):
                            th()
                        deferred = []
                        for ci in range(NI):
                            if ci + 1 < NI:
                                dma_x(ci + 1)
                            other = []
                            if ci + 1 < NI:
                                other.extend(qkv_thunks(ci + 1))
                            if ci - 1 >= 0:
                                pth = proj_thunks(ci - 1)
                                if ci < NI - 1:
                                    other.extend(pth[:4])
                                    deferred.extend(pth[4:])
                                else:
                                    other.extend(deferred)
                                    other.extend(pth)
                                    deferred = []
                            slots = attn_slots(ci)
                            n_s, n_o = len(slots), len(other)
                            oi = 0
                            for si, s in enumerate(slots):
                                s()
                                while (oi < n_o
                                       and (oi + 1) / n_o <= (si + 1) / n_s):
                                    other[oi]()
                                    oi += 1
                            while oi < n_o:
                                other[oi]()
                                oi += 1
                        for th in proj_thunks(NI - 1):
                            th()

                    def body_qkv():
                        for ci in range(NI):
                            dma_x(ci)
                        for ci in range(NI):
                            for th in qkv_thunks(ci):
                                th()

                    def body_attn():
                        for ci in range(NI):
                            for s in attn_slots(ci):
                                s()

                    def body_proj():
                        for ci in range(NI):
                            for th in proj_thunks(ci):
                                th()

                    if only is None:
                        body = body_full
                    elif only == "qkv":
                        body = body_qkv
                    elif only == "attn":
                        body_qkv()          # prelude: populate qkt/vaug once
                        body = body_attn
                    elif only == "proj":
                        body_qkv()
                        body_attn()
                        body = body_proj
                    else:
                        raise ValueError(only)

                    if reps == 1:
                        body()
                    else:
                        with tc.For_i(0, reps, 1, hint_engines=(
                                mybir.EngineType.PE,
                                mybir.EngineType.Activation,
                                mybir.EngineType.DVE, mybir.EngineType.SP,
                                mybir.EngineType.Pool)):
                            body()

    if fuse_ldw is None:
        fuse_ldw = _os.environ.get("KERNEL_LDW_OPT", "1") == "1"
    if fuse_ldw:
        fuse_ldweights(nc)
    if split_waits:
        split_excess_waits(nc)
    return nc


# ---------------------------------------------------------------------------
# Cached PJRT runner (fork of concourse.bass2jax.run_bass_via_pjrt that keeps
# the jitted executable so repeat kernel() calls don't recompile)
# ---------------------------------------------------------------------------
_RUNNERS = {}


def _make_pjrt(nc, donate=True, tag="main"):
    import jax
    from jax.sharding import Mesh, PartitionSpec
    from jax.experimental.shard_map import shard_map
    from concourse import bass2jax as b2j

    b2j.install_neuronx_cc_hook()

    partition_name = (
        nc.partition_id_tensor.name if nc.partition_id_tensor else None
    )
    in_names, out_names, out_avals, zero_outs = [], [], [], []
    for alloc in nc.m.functions[0].allocations:
        if not isinstance(alloc, mybir.MemoryLocationSet):
            continue
        name = alloc.memorylocations[0].name
        if alloc.kind == "ExternalInput":
            if name != partition_name:
                in_names.append(name)
        elif alloc.kind == "ExternalOutput":
            out_names.append(name)
            shape = tuple(alloc.tensor_shape)
            dtype = mybir.dt.np(alloc.dtype)
            out_avals.append(jax.core.ShapedArray(shape, dtype))
            zero_outs.append(np.zeros(shape, dtype))
    n_params = len(in_names)
    n_outs = len(out_avals)
    all_names = in_names + out_names
    if partition_name is not None:
        all_names = all_names + [partition_name]
    donate_idx = tuple(range(n_params, n_params + n_outs))

    def _body(*args):
        operands = list(args)
        if partition_name is not None:
            operands.append(b2j.partition_id_tensor())
        outs = b2j._bass_exec_p.bind(
            *operands,
            out_avals=tuple(out_avals),
            in_names=tuple(all_names),
            out_names=tuple(out_names),
            lowering_input_output_aliases=(),
            sim_require_finite=True,
            sim_require_nnan=True,
            nc=nc,
        )
        return tuple(outs)

    _body.__name__ = f"_body_{tag}"
    _body.__qualname__ = f"_body_{tag}"

    devices = jax.devices()[:N_CORES]
    mesh = Mesh(np.asarray(devices), ("core",))
    in_specs = (PartitionSpec("core"),) * (n_params + n_outs)
    out_specs = (PartitionSpec("core"),) * n_outs
    sharded = jax.jit(
        shard_map(_body, mesh=mesh, in_specs=in_specs, out_specs=out_specs,
                  check_rep=False),
        donate_argnums=donate_idx if donate else (), keep_unused=True)

    def concat_args(in_maps):
        per_core = [[np.asarray(m[name]) for name in in_names] for m in in_maps]
        concat_in = [
            np.concatenate([per_core[c][i] for c in range(N_CORES)], axis=0)
            for i in range(n_params)
        ]
        concat_zeros = [
            np.zeros((N_CORES * z.shape[0], *z.shape[1:]), z.dtype)
            for z in zero_outs
        ]
        return concat_in + concat_zeros

    def run(in_maps):
        out_arrs = sharded(*concat_args(in_maps))
        return [
            {name: np.asarray(out_arrs[i]).reshape(N_CORES, *out_avals[i].shape)[c]
             for i, name in enumerate(out_names)}
            for c in range(N_CORES)
        ]

    info = {
        "sharded": sharded, "concat_args": concat_args, "mesh": mesh,
        "PartitionSpec": PartitionSpec, "jax": jax,
    }
    return run, info


def _get_runner(key, nc):
    if key in _RUNNERS:
        return _RUNNERS[key]
    run, _ = _make_pjrt(nc, donate=True, tag=key)
    _RUNNERS[key] = run
    return run


def get_timed_runner(nc, tag="timed"):
    """No donation, device-resident args: returns (call, dev_args_fn)."""
    run, info = _make_pjrt(nc, donate=False, tag=tag)
    jax = info["jax"]
    sharding = jax.sharding.NamedSharding(
        info["mesh"], info["PartitionSpec"]("core"))

    def prepare(in_maps):
        return [jax.device_put(a, sharding) for a in info["concat_args"](in_maps)]

    def call(dev_args):
        outs = info["sharded"](*dev_args)
        jax.block_until_ready(outs)
        return outs

    return prepare, call


# ---------------------------------------------------------------------------
# Host-side sharding / gathering
# ---------------------------------------------------------------------------
def _make_masks():
    import ml_dtypes
    rj = np.arange(128)[:, None]
    ri = np.arange(128)[None, :]
    tri = (rj <= ri).astype(ml_dtypes.bfloat16)
    return np.ascontiguousarray(np.concatenate([tri, tri], axis=1))


def make_in_maps(x, W_attn, b_attn, W_proj):
    import ml_dtypes
    masks = _make_masks()
    use_bias = bool(np.any(b_attn != 0))
    in_maps = []
    for c in range(N_CORES):
        b = c // (N_CORES // B)
        g = c % (N_CORES // B)
        cs = slice(CPH * g, CPH * g + CPH)
        wq = W_attn[:, CPH * g:CPH * g + CPH]
        wk = W_attn[:, C + CPH * g:C + CPH * g + CPH]
        wv = W_attn[:, 2 * C + CPH * g:2 * C + CPH * g + CPH]
        m = {
            "xt": np.ascontiguousarray(x[b].T.astype(ml_dtypes.bfloat16)),
            "wqk": np.ascontiguousarray(
                np.concatenate([wq, wk], axis=1).astype(ml_dtypes.bfloat16)),
            "wv": np.ascontiguousarray(wv.astype(ml_dtypes.bfloat16)),
            "wp": np.ascontiguousarray(W_proj[cs, :].astype(ml_dtypes.bfloat16)),
            "masks": masks,
        }
        if use_bias:
            w9 = np.zeros((128, 768), dtype=ml_dtypes.bfloat16)
            w9[0, 0:256] = b_attn[CPH * g:CPH * g + CPH]
            w9[0, 256:512] = b_attn[C + CPH * g:C + CPH * g + CPH]
            w9[0, 512:768] = b_attn[2 * C + CPH * g:2 * C + CPH * g + CPH]
            m["w9"] = w9
        in_maps.append(m)
    return in_maps


def kernel(x, W_attn, b_attn, W_proj, b_proj):
    x = np.asarray(x, dtype=np.float32)
    W_attn = np.asarray(W_attn, dtype=np.float32)
    b_attn = np.asarray(b_attn, dtype=np.float32)
    W_proj = np.asarray(W_proj, dtype=np.float32)
    b_proj = np.asarray(b_proj, dtype=np.float32)

    use_bias = bool(np.any(b_attn != 0))
    key = f"main_bias{int(use_bias)}"
    if key not in _RUNNERS:
        nc = build_program(reps=1, use_bias=use_bias)
        run = _get_runner(key, nc)
    else:
        run = _RUNNERS[key]

    results = run(make_in_maps(x, W_attn, b_attn, W_proj))

    out = np.empty((B, T, C), dtype=np.float32)
    gpb = N_CORES // B
    for b in range(B):
        acc = results[gpb * b]["y"].astype(np.float32).copy()
        for g in range(1, gpb):
            acc += results[gpb * b + g]["y"]
        out[b] = acc + b_proj[None, :]
    return out


# revision 38
# speedup vs baseline: 1.0111x; 1.0111x over previous
"""Causal self-attention (B=2, T=2048, C=1024, H=16) on 8 TRN2 NeuronCores.

Sharding: data-parallel over batch x tensor-parallel over heads.
Core c handles batch c//4 and the 4 heads (c%4)*4 .. (c%4)*4+3.

v2 design (vs v1 baseline):
  - x is transposed on the host; the kernel receives x^T [C, T] directly,
    eliminating all PE transposes and their PSUM->SBUF evacuations.
  - The per-rep schedule is a software pipeline over 512-row chunks ci:
    QKV(ci+1) and PROJ(ci-1) matmul groups are interleaved into the
    ATTN(ci) instruction stream so the PE never drains while the scalar
    engine works through the exp()s (attention is ACT-bound per block).
  - ST matmuls for the two heads of a pair sit in disjoint 64-row groups
    of the PE array (tile_position auto-derived from base_partition) and
    are issued back-to-back, so they execute concurrently on hardware.
  - exp() runs once per (pair, j-block) on a [128, 2x512] PSUM region via
    a 3D access pattern, halving activation-instruction overhead.
  - The causal mask multiplies only the 128-col diagonal triangle block,
    in place, with a host-precomputed doubled [tri|tri] tile.
  - The softmax-denominator ones-column of V_aug is memset once at setup;
    the attn bias (zero in this problem) folds into an extra contraction
    block only when nonzero.
  - Explicit Ldweights instructions are folded back into self-loading
    matmuls (fuse_ldweights) and walrus ldw-opt is re-enabled via a
    driver shim, shrinking the PE instruction stream.

Matmuls run in bf16 / float32r (~1.5e-4 rel err, 4x the fp32 rate).
"""
import sys
sys.path.insert(0, '/opt/trn_rl_repo')

from contextlib import ExitStack

import numpy as np

import concourse.bass as bass
import concourse.tile as tile
from concourse import mybir

B, T, C, H, HD = 2, 2048, 1024, 16, 64
N_CORES = 8
HPC = H // (N_CORES // B)     # heads per core = 4
CPH = HPC * HD                # channel slice per core = 256

f32 = mybir.dt.float32
f32r = mybir.dt.float32r
bf16 = mybir.dt.bfloat16
AF = mybir.ActivationFunctionType

NI = T // 512   # 4 chunks of 512 rows
NCB = C // 128  # 8 contraction blocks

# ---------------------------------------------------------------------------
# Workaround for this container's walrus codegen, which rejects instructions
# carrying more than one sync-wait command ("Too many sync wait commands").
# After Tile scheduling, hoist excess waits onto same-engine NoOps inserted
# immediately before the owning instruction (engine streams are sequential,
# so this preserves semantics exactly).
# ---------------------------------------------------------------------------
import concourse.tile as tile_mod
from bass_rust import ScopedClock, SyncInfo

MAX_WAITS = 1


def _drain_and_barrier(self, tick_clock, wait_clock):
    nc = self.nc
    drain_inst = nc.sync.drain()
    wait_clock.add_sem_waits(
        drain_inst.ins, ScopedClock({None: tick_clock.global_clock})
    )
    si = drain_inst.ins.sync_info
    if si is not None and len(si.on_wait) > MAX_WAITS:
        waits = list(si.on_wait)
        drain_inst.ins.sync_info = SyncInfo(
            on_wait=waits[:MAX_WAITS], on_update=list(si.on_update)
        )
        for k in range(MAX_WAITS, len(waits), MAX_WAITS):
            nop = nc.sync.nop(nofuse=True)
            nop.ins.sync_info = SyncInfo(on_wait=waits[k:k + MAX_WAITS], on_update=[])
    nc.all_engine_barrier()
    assert self.sems is not None
    popped = nc._tile_sem_poison_stack.pop()
    assert popped is self._sem_poison
    nc.clear_and_free_semaphores(list(self.sems.allocated().values()))
    nc.all_engine_barrier()


tile_mod.TileContext._drain_and_barrier = _drain_and_barrier

# ---------------------------------------------------------------------------
# The stock bass compile path hardcodes --enable-ldw-opt=false.  With the
# explicit Ldweights folded into self-loading matmuls (fuse_ldweights
# below), re-enabling ldw-opt lets walrus elide repeated identical weight
# loads (verified by NEFF disasm) and halves the PE instruction count.
# Controlled by KERNEL_LDW_OPT (default on) for A/B measurement.
# ---------------------------------------------------------------------------
import os as _os
import stat as _stat

_LDWOPT_SENTINEL = "/tmp/ldwopt_on"

if _os.environ.get("KERNEL_LDW_OPT", "1") == "1":
    import concourse.bass_utils as _bass_utils_mod

    if getattr(_bass_utils_mod.get_walrus_driver, "_is_ldw_shim", False):
        _orig_get_walrus_driver = _bass_utils_mod.get_walrus_driver._orig
    else:
        _orig_get_walrus_driver = _bass_utils_mod.get_walrus_driver

    def _shimmed_walrus_driver():
        real = _orig_get_walrus_driver()
        shim = "/tmp/walrus_ldwopt_shim.sh"
        with open(shim, "w") as f:
            f.write(
                "#!/bin/sh\n"
                f'if [ -f {_LDWOPT_SENTINEL} ]; then\n'
                'args=""\n'
                'for a in "$@"; do\n'
                '  case "$a" in\n'
                "    --enable-ldw-opt=false) a=--enable-ldw-opt=true;;\n"
                "  esac\n"
                '  args="$args \\"$a\\""\n'
                "done\n"
                f'eval exec "{real}" $args\n'
                "else\n"
                f'exec "{real}" "$@"\n'
                "fi\n"
            )
        _os.chmod(shim, _os.stat(shim).st_mode | _stat.S_IEXEC)
        return shim

    _shimmed_walrus_driver._is_ldw_shim = True
    _shimmed_walrus_driver._orig = _orig_get_walrus_driver
    _bass_utils_mod.get_walrus_driver = _shimmed_walrus_driver


def ldwopt_on():
    with open(_LDWOPT_SENTINEL, "w") as f:
        f.write("1")


def ldwopt_off():
    if _os.path.exists(_LDWOPT_SENTINEL):
        _os.remove(_LDWOPT_SENTINEL)


if _os.environ.get("KERNEL_LDW_OPT", "1") == "1":
    ldwopt_on()

_split_counter = [0]


def fuse_ldweights(nc):
    """Fold explicit InstLdweights into their Matmult (self-loading form).

    The tile scheduler splits each matmul into Ldweights+Matmult, but this
    container's walrus runs with LDW optimization rejected for explicit
    Ldweights, so every weight load serializes with the matmul stream
    (~100ns each).  Deleting the Ldweights and setting ldweights=True on the
    Matmult hands weight-load scheduling back to walrus codegen, whose
    ldw-opt (re-enabled via the driver shim) pipelines loads into the
    background weight buffer.  The Ldweights' sem waits/updates move onto
    the Matmult: waits still gate the weight read; updates fire at matmul
    completion, which only lengthens the protected window (safe).
    """
    n = 0
    for f in nc.m.functions:
        for bb in f.blocks:
            il = bb.instructions
            out = []
            pending = None
            for ins in il:
                if ins.opcode == "Ldweights":
                    assert pending is None, "two Ldweights without a Matmult"
                    pending = ins
                    continue
                if (pending is not None
                        and ins.engine == mybir.EngineType.PE
                        and ins.opcode != "Matmult"):
                    # unexpected PE instruction between the pair: keep the
                    # explicit Ldweights rather than fusing across it
                    out.append(pending)
                    pending = None
                if (pending is not None
                        and ins.engine == mybir.EngineType.PE):
                    lsi, msi = pending.sync_info, ins.sync_info
                    waits = list(lsi.on_wait) if lsi else []
                    upds = list(lsi.on_update) if lsi else []
                    if msi is not None:
                        waits += list(msi.on_wait)
                        upds += list(msi.on_update)
                    ins.sync_info = SyncInfo(on_wait=waits, on_update=upds)
                    ins.ldweights = True
                    pending = None
                    n += 1
                out.append(ins)
            if pending is not None:
                out.append(pending)
            if len(out) != len(il):
                il[:] = out
    return n


def split_excess_waits(nc, max_waits=MAX_WAITS):
    n_split = 0
    for f in nc.m.functions:
        for bb in f.blocks:
            il = bb.instructions
            out = []
            for ins in il:
                si = ins.sync_info
                if si is not None and len(si.on_wait) > max_waits:
                    waits = list(si.on_wait)
                    extra = waits[:-max_waits]
                    for k in range(0, len(extra), max_waits):
                        _split_counter[0] += 1
                        nop = mybir.InstNoOp(
                            name=f"wsplit-{_split_counter[0]}", ins=[], outs=[]
                        )
                        nop.engine = ins.engine
                        nop.sync_info = SyncInfo(
                            on_wait=extra[k:k + max_waits], on_update=[]
                        )
                        out.append(nop)
                    ins.sync_info = SyncInfo(
                        on_wait=waits[-max_waits:], on_update=list(si.on_update)
                    )
                    n_split += 1
                out.append(ins)
            if len(out) != len(il):
                il[:] = out
    return n_split


# ---------------------------------------------------------------------------
# Program builder
# ---------------------------------------------------------------------------
def build_program(reps=1, split_waits=True, use_bias=False, only=None,
                  fuse_ldw=None):
    nc = bass.Bass("TRN2", target_bir_lowering=False, debug=False)

    xt_d = nc.dram_tensor("xt", [C, T], bf16, kind="ExternalInput")
    wqk_d = nc.dram_tensor("wqk", [C, 512], bf16, kind="ExternalInput")
    wv_d = nc.dram_tensor("wv", [C, CPH], bf16, kind="ExternalInput")
    wp_d = nc.dram_tensor("wp", [CPH, C], bf16, kind="ExternalInput")
    masks_d = nc.dram_tensor("masks", [128, 256], bf16, kind="ExternalInput")
    if use_bias:
        w9_d = nc.dram_tensor("w9", [128, 768], bf16, kind="ExternalInput")
    y_d = nc.dram_tensor("y", [T, C], bf16, kind="ExternalOutput")

    with tile.TileContext(nc) as tc:
        with ExitStack() as ctx:
            const = ctx.enter_context(tc.tile_pool(name="const", bufs=1))
            tri2 = const.tile([128, 256], bf16, tag="tri2")
            nc.sync.dma_start(tri2[:], masks_d.ap())
            # ones row for broadcasting reciprocal rows across partitions
            tones_f = const.tile([1, 64], f32, tag="tones_f")
            nc.gpsimd.memset(tones_f[:], 1.0)
            tones = const.tile([1, 64], f32r, tag="tones")
            nc.vector.tensor_copy(tones[:], tones_f[:])
            if use_bias:
                x9 = const.tile([128, 512], bf16, tag="x9")
                nc.gpsimd.memset(x9[0:1, :], 1.0)
                nc.gpsimd.memset(x9[1:128, :], 0.0)
                w9_t = const.tile([128, 768], bf16, tag="w9")
                nc.sync.dma_start(w9_t[:], w9_d.ap())

            # persistent data tiles (reused every rep)
            pers = ctx.enter_context(tc.tile_pool(name="pers", bufs=1))
            qkt = [pers.tile([128, T], bf16, tag=f"qkt{m}", name=f"qkt{m}")
                   for m in range(4)]
            vaug = [pers.tile([128, HPC * 65], bf16, tag=f"va{tb}", name=f"va{tb}")
                    for tb in range(T // 128)]
            for tb in range(T // 128):
                nc.gpsimd.memset(
                    vaug[tb][:].rearrange("p (h e) -> p h e", e=65)[:, :, 64:65],
                    1.0)
            yts = [pers.tile([128, T], bf16, tag=f"yts{k}", name=f"yts{k}")
                   for k in range(2)]
            wpt = [pers.tile([128, C], bf16, tag=f"wp{kb}", name=f"wpt{kb}")
                   for kb in range(2)]
            wqk_t = [pers.tile([128, 512], bf16, tag=f"wqk{cb}", name=f"wqk{cb}")
                     for cb in range(NCB)]
            wv_t = [pers.tile([128, CPH], bf16, tag=f"wv{cb}", name=f"wv{cb}")
                    for cb in range(NCB)]

            with ExitStack() as c2:
                    xp = c2.enter_context(tc.tile_pool(name="xp", bufs=2))
                    mmps = c2.enter_context(
                        tc.tile_pool(name="mmps", bufs=2, space="PSUM"))
                    sps = c2.enter_context(
                        tc.tile_pool(name="sps", bufs=2, space="PSUM"))
                    yps = c2.enter_context(
                        tc.tile_pool(name="yps", bufs=2, space="PSUM"))
                    ep = c2.enter_context(tc.tile_pool(name="ep", bufs=3))
                    rp = c2.enter_context(tc.tile_pool(name="rp", bufs=2))
                    op = c2.enter_context(tc.tile_pool(name="op", bufs=3))

                    # per-rep input DMAs
                    xts = {}

                    def dma_x(ci):
                        ts = []
                        for cb in range(NCB):
                            t = xp.tile([128, 512], bf16, tag=f"x{cb}", name=f"x{ci}_{cb}")
                            nc.sync.dma_start(
                                t[:],
                                xt_d.ap()[cb * 128:(cb + 1) * 128,
                                          ci * 512:(ci + 1) * 512])
                            ts.append(t)
                        xts[ci] = ts

                    # weights are constant across repeat-iterations: load
                    # them once at setup instead of every rep (removes the
                    # rep-boundary DMA serialization).
                    for cb in range(NCB):
                        nc.sync.dma_start(
                            wqk_t[cb][:],
                            wqk_d.ap()[cb * 128:(cb + 1) * 128, :])
                    for cb in range(NCB):
                        nc.sync.dma_start(
                            wv_t[cb][:],
                            wv_d.ap()[cb * 128:(cb + 1) * 128, :])
                    for kb in range(2):
                        nc.sync.dma_start(
                            wpt[kb][:],
                            wp_d.ap()[kb * 128:(kb + 1) * 128, :])

                    # ---- QKV for chunk ci: list of emit-thunks ------------
                    def qkv_thunks(ci):
                        thunks = []

                        def qk_group(m):
                            def emit():
                                ps = mmps.tile([128, 512], f32, tag="mm", name=f"qk{ci}_{m}")
                                for cb in range(NCB):
                                    nc.tensor.matmul(
                                        ps[:],
                                        lhsT=wqk_t[cb][:, m * 128:(m + 1) * 128],
                                        rhs=xts[ci][cb][:],
                                        start=(cb == 0),
                                        stop=(cb == NCB - 1 and not use_bias))
                                if use_bias:
                                    nc.tensor.matmul(
                                        ps[:],
                                        lhsT=w9_t[:, m * 128:(m + 1) * 128],
                                        rhs=x9[:],
                                        start=False, stop=True)
                                nc.vector.tensor_copy(
                                    qkt[m][:, ci * 512:(ci + 1) * 512], ps[:])
                            return emit

                        for m in range(4):
                            thunks.append(qk_group(m))

                        def v_group(tp_):
                            def emit():
                                ps = mmps.tile([128, 512], f32, tag="mm", name=f"v{ci}_{tp_}")
                                for sub in range(2):
                                    tloc = 2 * tp_ + sub
                                    for cb in range(NCB):
                                        nc.tensor.matmul(
                                            ps[:, sub * 256:(sub + 1) * 256],
                                            lhsT=xts[ci][cb][:, tloc * 128:
                                                             (tloc + 1) * 128],
                                            rhs=wv_t[cb][:],
                                            start=(cb == 0),
                                            stop=(cb == NCB - 1 and not use_bias))
                                    if use_bias:
                                        nc.tensor.matmul(
                                            ps[:, sub * 256:(sub + 1) * 256],
                                            lhsT=x9[:, 0:128],
                                            rhs=w9_t[:, 512:768],
                                            start=False, stop=True)
                                for sub in range(2):
                                    tb = 4 * ci + 2 * tp_ + sub
                                    nc.vector.tensor_copy(
                                        vaug[tb][:].rearrange(
                                            "p (h e) -> p h e", e=65)[:, :, 0:64],
                                        ps[:, sub * 256:(sub + 1) * 256].rearrange(
                                            "p (h d) -> p h d", d=64))
                            return emit

                        for tp_ in range(2):
                            thunks.append(v_group(tp_))
                        return thunks

                    # ---- PROJ for chunk ci: list of emit-thunks -----------
                    def proj_thunks(ci):
                        thunks = []

                        def pgroup(tb, nn_):
                            def emit():
                                ps = mmps.tile([128, 512], f32, tag="mm",
                                               name=f"p{ci}_{tb}_{nn_}")
                                for kb in range(2):
                                    nc.tensor.matmul(
                                        ps[:],
                                        lhsT=yts[kb][:, tb * 128:(tb + 1) * 128],
                                        rhs=wpt[kb][:, nn_ * 512:(nn_ + 1) * 512],
                                        start=(kb == 0), stop=(kb == 1))
                                ob = op.tile([128, 512], bf16, tag="ob", name=f"ob{tb}_{nn_}")
                                nc.vector.tensor_copy(ob[:], ps[:])
                                nc.sync.dma_start(
                                    y_d.ap()[tb * 128:(tb + 1) * 128,
                                             nn_ * 512:(nn_ + 1) * 512],
                                    ob[:])
                            return emit

                        for tb in range(4 * ci, 4 * ci + 4):
                            for nn_ in range(2):
                                thunks.append(pgroup(tb, nn_))
                        return thunks

                    # ---- ATTN for chunk ci: list of slot-thunks -----------
                    # The two head-pairs run as interleaved independent
                    # chains (pair0-bj, pair1-bj, pair0-bj+1, ...): doubles
                    # the dependency distance of the ST->exp->mask->PV chain
                    # so cross-engine semaphore latency is hidden, with the
                    # same PSUM budget (st 1x2banks, yt 4x1bank).
                    def attn_slots(ci):
                        isl = slice(ci * 512, ci * 512 + 512)
                        jmax = 4 * ci + 3
                        state = {0: {}, 1: {}}

                        def bj_slot(pair, bj):
                            qt = qkt[pair]
                            kt = qkt[2 + pair]
                            hA, hB = 2 * pair, 2 * pair + 1

                            def emit():
                                if bj == 0:
                                    state[pair]["ytA"] = yps.tile(
                                        [128, 512], f32, tag="yt",
                                        name=f"ytA{ci}_{pair}")
                                    state[pair]["ytB"] = yps.tile(
                                        [128, 512], f32, tag="yt",
                                        name=f"ytB{ci}_{pair}")
                                ytA = state[pair]["ytA"]
                                ytB = state[pair]["ytB"]
                                jsl = slice(bj * 128, bj * 128 + 128)
                                k = bj - 4 * ci
                                lo = max(k, 0) * 128
                                st = sps.tile([128, 1024], f32, tag="st",
                                              name=f"st{ci}_{pair}_{bj}")
                                nc.tensor.matmul(
                                    st[:, lo:512], lhsT=kt[0:64, jsl],
                                    rhs=qt[0:64, isl][:, lo:],
                                    start=True, stop=True)
                                nc.tensor.matmul(
                                    st[:, 512 + lo:1024], lhsT=kt[64:128, jsl],
                                    rhs=qt[64:128, isl][:, lo:],
                                    start=True, stop=True)
                                es = ep.tile([128, 1024], bf16, tag="es",
                                             name=f"es{ci}_{pair}_{bj}")
                                st3 = st[:].rearrange(
                                    "p (c i) -> p c i", c=2)[:, :, lo:]
                                es3 = es[:].rearrange(
                                    "p (c i) -> p c i", c=2)[:, :, lo:]
                                nc.scalar.activation(es3, st3, AF.Exp,
                                                     scale=0.125)
                                if k >= 0:
                                    es_tri = es[:].rearrange(
                                        "p (c i) -> p c i",
                                        c=2)[:, :, lo:lo + 128]
                                    nc.vector.tensor_mul(
                                        es_tri, es_tri,
                                        tri2[:].rearrange(
                                            "p (c i) -> p c i", c=2))
                                nc.tensor.matmul(
                                    ytA[0:65, lo:],
                                    lhsT=vaug[bj][:, hA * 65:(hA + 1) * 65],
                                    rhs=es[:, lo:512],
                                    start=(bj == 0), stop=(bj == jmax))
                                nc.tensor.matmul(
                                    ytB[0:65, lo:],
                                    lhsT=vaug[bj][:, hB * 65:(hB + 1) * 65],
                                    rhs=es[:, 512 + lo:1024],
                                    start=(bj == 0), stop=(bj == jmax))
                            return emit

                        def norm_slot(pair):
                            def emit():
                                ytA = state[pair]["ytA"]
                                ytB = state[pair]["ytB"]
                                for sub, yt_h in ((0, ytA), (1, ytB)):
                                    rc = rp.tile([1, 512], f32r, tag="rc",
                                                 name=f"rc{ci}_{pair}_{sub}")
                                    with nc.allow_low_precision(
                                            reason="f32r operand for bc mm"):
                                        nc.vector.reciprocal(rc[:],
                                                             yt_h[64:65, :])
                                    bc = mmps.tile([128, 512], f32, tag="mm",
                                                   name=f"bc{ci}_{pair}_{sub}")
                                    nc.tensor.matmul(bc[0:64, :],
                                                     lhsT=tones[:], rhs=rc[:],
                                                     start=True, stop=True)
                                    bs = rp.tile([64, 512], f32, tag="bs",
                                                 name=f"bs{ci}_{pair}_{sub}")
                                    nc.vector.tensor_copy(bs[:], bc[0:64, :])
                                    prow = slice(sub * 64, sub * 64 + 64)
                                    nc.vector.tensor_mul(
                                        yts[pair][prow, isl], yt_h[0:64, :],
                                        bs[:])
                            return emit

                        slots = []
                        for pair in range(2):
                            for bj in range(jmax + 1):
                                slots.append(bj_slot(pair, bj))
                            slots.append(norm_slot(pair))
                        return slots

                    # ---- emission schedules -------------------------------
                    def body_full():
                        dma_x(0)
                        for th in qkv_thunks(0):
                            th()
                        deferred = []
                        for ci in range(NI):
                            if ci + 1 < NI:
                                dma_x(ci + 1)
                            other = []
                            if ci + 1 < NI:
                                other.extend(qkv_thunks(ci + 1))
                            if ci - 1 >= 0:
                                pth = proj_thunks(ci - 1)
                                if ci < NI - 1:
                                    other.extend(pth[:4])
                                    deferred.extend(pth[4:])
                                else:
                                    other.extend(deferred)
                                    other.extend(pth)
                                    deferred = []
                            slots = attn_slots(ci)
                            n_s, n_o = len(slots), len(other)
                            oi = 0
                            for si, s in enumerate(slots):
                                s()
                                while (oi < n_o
                                       and (oi + 1) / n_o <= (si + 1) / n_s):
                                    other[oi]()
                                    oi += 1
                            while oi < n_o:
                                other[oi]()
                                oi += 1
                        for th in proj_thunks(NI - 1):
                            th()

                    def body_qkv():
                        for ci in range(NI):
                            dma_x(ci)
                        for ci in range(NI):
                            for th in qkv_thunks(ci):
                                th()

                    def body_attn():
                        for ci in range(NI):
                            for s in attn_slots(ci):
                                s()

                    def body_proj():
                        for ci in range(NI):
                            for th in proj_thunks(ci):
                                th()

                    if only is None:
                        body = body_full
                    elif only == "qkv":
                        body = body_qkv
                    elif only == "attn":
                        body_qkv()          # prelude: populate qkt/vaug once
                        body = body_attn
                    elif only == "proj":
                        body_qkv()
                        body_attn()
                        body = body_proj
                    else:
                        raise ValueError(only)

                    if reps == 1:
                        body()
                    else:
                        with tc.For_i(0, reps, 1, hint_engines=(
                                mybir.EngineType.PE,
                                mybir.EngineType.Activation,
                                mybir.EngineType.DVE, mybir.EngineType.SP,
                                mybir.EngineType.Pool)):
                            body()

    if fuse_ldw is None:
        fuse_ldw = _os.environ.get("KERNEL_LDW_OPT", "1") == "1"
    if fuse_ldw:
        fuse_ldweights(nc)
    if split_waits:
        split_excess_waits(nc)
    return nc


# ---------------------------------------------------------------------------
# Cached PJRT runner (fork of concourse.bass2jax.run_bass_via_pjrt that keeps
# the jitted executable so repeat kernel() calls don't recompile)
# ---------------------------------------------------------------------------
_RUNNERS = {}


def _make_pjrt(nc, donate=True, tag="main"):
    import jax
    from jax.sharding import Mesh, PartitionSpec
    from jax.experimental.shard_map import shard_map
    from concourse import bass2jax as b2j

    b2j.install_neuronx_cc_hook()

    partition_name = (
        nc.partition_id_tensor.name if nc.partition_id_tensor else None
    )
    in_names, out_names, out_avals, zero_outs = [], [], [], []
    for alloc in nc.m.functions[0].allocations:
        if not isinstance(alloc, mybir.MemoryLocationSet):
            continue
        name = alloc.memorylocations[0].name
        if alloc.kind == "ExternalInput":
            if name != partition_name:
                in_names.append(name)
        elif alloc.kind == "ExternalOutput":
            out_names.append(name)
            shape = tuple(alloc.tensor_shape)
            dtype = mybir.dt.np(alloc.dtype)
            out_avals.append(jax.core.ShapedArray(shape, dtype))
            zero_outs.append(np.zeros(shape, dtype))
    n_params = len(in_names)
    n_outs = len(out_avals)
    all_names = in_names + out_names
    if partition_name is not None:
        all_names = all_names + [partition_name]
    donate_idx = tuple(range(n_params, n_params + n_outs))

    def _body(*args):
        operands = list(args)
        if partition_name is not None:
            operands.append(b2j.partition_id_tensor())
        outs = b2j._bass_exec_p.bind(
            *operands,
            out_avals=tuple(out_avals),
            in_names=tuple(all_names),
            out_names=tuple(out_names),
            lowering_input_output_aliases=(),
            sim_require_finite=True,
            sim_require_nnan=True,
            nc=nc,
        )
        return tuple(outs)

    _body.__name__ = f"_body_{tag}"
    _body.__qualname__ = f"_body_{tag}"

    devices = jax.devices()[:N_CORES]
    mesh = Mesh(np.asarray(devices), ("core",))
    in_specs = (PartitionSpec("core"),) * (n_params + n_outs)
    out_specs = (PartitionSpec("core"),) * n_outs
    sharded = jax.jit(
        shard_map(_body, mesh=mesh, in_specs=in_specs, out_specs=out_specs,
                  check_rep=False),
        donate_argnums=donate_idx if donate else (), keep_unused=True)

    def concat_args(in_maps):
        per_core = [[np.asarray(m[name]) for name in in_names] for m in in_maps]
        concat_in = [
            np.concatenate([per_core[c][i] for c in range(N_CORES)], axis=0)
            for i in range(n_params)
        ]
        concat_zeros = [
            np.zeros((N_CORES * z.shape[0], *z.shape[1:]), z.dtype)
            for z in zero_outs
        ]
        return concat_in + concat_zeros

    def run(in_maps):
        out_arrs = sharded(*concat_args(in_maps))
        return [
            {name: np.asarray(out_arrs[i]).reshape(N_CORES, *out_avals[i].shape)[c]
             for i, name in enumerate(out_names)}
            for c in range(N_CORES)
        ]

    info = {
        "sharded": sharded, "concat_args": concat_args, "mesh": mesh,
        "PartitionSpec": PartitionSpec, "jax": jax,
    }
    return run, info


def _get_runner(key, nc):
    if key in _RUNNERS:
        return _RUNNERS[key]
    run, _ = _make_pjrt(nc, donate=True, tag=key)
    _RUNNERS[key] = run
    return run


def get_timed_runner(nc, tag="timed"):
    """No donation, device-resident args: returns (call, dev_args_fn)."""
    run, info = _make_pjrt(nc, donate=False, tag=tag)
    jax = info["jax"]
    sharding = jax.sharding.NamedSharding(
        info["mesh"], info["PartitionSpec"]("core"))

    def prepare(in_maps):
        return [jax.device_put(a, sharding) for a in info["concat_args"](in_maps)]

    def call(dev_args):
        outs = info["sharded"](*dev_args)
        jax.block_until_ready(outs)
        return outs

    return prepare, call


# ---------------------------------------------------------------------------
# Host-side sharding / gathering
# ---------------------------------------------------------------------------
def _make_masks():
    import ml_dtypes
    rj = np.arange(128)[:, None]
    ri = np.arange(128)[None, :]
    tri = (rj <= ri).astype(ml_dtypes.bfloat16)
    return np.ascontiguousarray(np.concatenate([tri, tri], axis=1))


def make_in_maps(x, W_attn, b_attn, W_proj):
    import ml_dtypes
    masks = _make_masks()
    use_bias = bool(np.any(b_attn != 0))
    in_maps = []
    for c in range(N_CORES):
        b = c // (N_CORES // B)
        g = c % (N_CORES // B)
        cs = slice(CPH * g, CPH * g + CPH)
        wq = W_attn[:, CPH * g:CPH * g + CPH]
        wk = W_attn[:, C + CPH * g:C + CPH * g + CPH]
        wv = W_attn[:, 2 * C + CPH * g:2 * C + CPH * g + CPH]
        m = {
            "xt": np.ascontiguousarray(x[b].T.astype(ml_dtypes.bfloat16)),
            "wqk": np.ascontiguousarray(
                np.concatenate([wq, wk], axis=1).astype(ml_dtypes.bfloat16)),
            "wv": np.ascontiguousarray(wv.astype(ml_dtypes.bfloat16)),
            "wp": np.ascontiguousarray(W_proj[cs, :].astype(ml_dtypes.bfloat16)),
            "masks": masks,
        }
        if use_bias:
            w9 = np.zeros((128, 768), dtype=ml_dtypes.bfloat16)
            w9[0, 0:256] = b_attn[CPH * g:CPH * g + CPH]
            w9[0, 256:512] = b_attn[C + CPH * g:C + CPH * g + CPH]
            w9[0, 512:768] = b_attn[2 * C + CPH * g:2 * C + CPH * g + CPH]
            m["w9"] = w9
        in_maps.append(m)
    return in_maps


def kernel(x, W_attn, b_attn, W_proj, b_proj):
    x = np.asarray(x, dtype=np.float32)
    W_attn = np.asarray(W_attn, dtype=np.float32)
    b_attn = np.asarray(b_attn, dtype=np.float32)
    W_proj = np.asarray(W_proj, dtype=np.float32)
    b_proj = np.asarray(b_proj, dtype=np.float32)

    use_bias = bool(np.any(b_attn != 0))
    key = f"main_bias{int(use_bias)}"
    if key not in _RUNNERS:
        nc = build_program(reps=1, use_bias=use_bias)
        run = _get_runner(key, nc)
    else:
        run = _RUNNERS[key]

    results = run(make_in_maps(x, W_attn, b_attn, W_proj))

    out = np.empty((B, T, C), dtype=np.float32)
    gpb = N_CORES // B
    for b in range(B):
        acc = results[gpb * b]["y"].astype(np.float32).copy()
        for g in range(1, gpb):
            acc += results[gpb * b + g]["y"]
        out[b] = acc + b_proj[None, :]
    return out
